# revision 1
# baseline (speedup 1.0000x reference)
"""BiLevelRoutingAttention TRN2 Bass kernel — redesigned (v3, pipelined).

Data-parallel over batch (16 imgs -> 8 cores x 2). Per image:
  - router: f32 pooling + tiny GEMM + top-4 threshold mask; mask transposed
    once via PE identity; per-head multiplicative masked exp-bias tiles.
  - qkv GEMMs in bf16 with the raster->window reorder folded into the
    moving-operand access patterns (no separate x rearrange pass).
  - attention computed TRANSPOSED: L^T[k_win, q_win] per (head, t) directly
    from natural GEMM-output slices; QK' tiles packed DENSE (16 per PSUM
    bank); exp fused into the PSUM drain; mask+bias+block-diag scatter as
    two 4x-mode multiplies; denominator via block-ones matmuls; reciprocal
    expanded with pick-matrix matmuls; normalization fused into the AV
    PSUM drain (tensor_tensor).
  - AV 2-t-packed against block-diagonal A (no A transposes at all).
  - output projection in bf16, raster reorder fused into drain APs.
  - emission is software-pipelined: attention(i) interleaves GEMM(i+1)
    chunks and proj(i-1) blocks so the PE never waits on drain chains.
No collectives (pure batch parallelism).
"""
import numpy as np

import concourse.bass as bass
import concourse.tile as tile
import concourse.mybir as mybir
from concourse import bacc
from concourse.bass_utils import run_bass_kernel_spmd
from concourse.masks import make_identity

F32 = mybir.dt.float32
BF16 = mybir.dt.bfloat16
AL = mybir.AluOpType
ACTF = mybir.ActivationFunctionType
AX = mybir.AxisListType

N_CORES = 8
B_FULL = 16
NB = B_FULL // N_CORES
C = 512
H = W = 56
HW = 3136
NWIN_SIDE = 7
NW = 49
HEADS = 16
SCALE = 32.0 ** -0.5


def build_nc2(nb=NB):
    nc = bacc.Bacc(None, target_bir_lowering=False, debug=False)

    xd = nc.dram_tensor("x", [nb, C, HW], F32, kind="ExternalInput")
    xbd = nc.dram_tensor("x_bf", [nb, C, HW], BF16, kind="ExternalInput")
    qkvw = nc.dram_tensor("qkv_wTb", [C, 3 * C], BF16, kind="ExternalInput")
    rw = nc.dram_tensor("r_wT", [C, NW], F32, kind="ExternalInput")
    rb = nc.dram_tensor("router_b", [1, NW], F32, kind="ExternalInput")
    pw = nc.dram_tensor("proj_wTb", [C, C], BF16, kind="ExternalInput")
    pb = nc.dram_tensor("proj_b", [1, C], F32, kind="ExternalInput")
    ebd = nc.dram_tensor("expbT2", [128, HEADS * NW], BF16,
                         kind="ExternalInput")
    cbd = nc.dram_tensor("constsb", [128, 260], BF16, kind="ExternalInput")
    yd = nc.dram_tensor("y", [nb, C, HW], F32, kind="ExternalOutput")

    from contextlib import ExitStack
    with tile.TileContext(nc) as tc, ExitStack() as ctx:
        wp = ctx.enter_context(tc.tile_pool(name="weights", bufs=1))
        xp = ctx.enter_context(tc.tile_pool(name="xin", bufs=2))
        xrp = ctx.enter_context(tc.tile_pool(name="xraster", bufs=2))
        xsp = ctx.enter_context(tc.tile_pool(name="xstage", bufs=2))
        qkp = ctx.enter_context(tc.tile_pool(name="qk", bufs=2))
        vtp = ctx.enter_context(tc.tile_pool(name="vt", bufs=1))
        vsp = ctx.enter_context(tc.tile_pool(name="vstg", bufs=2))
        ap_ = ctx.enter_context(tc.tile_pool(name="attn", bufs=3))
        rp = ctx.enter_context(tc.tile_pool(name="rden", bufs=2))
        op_ = ctx.enter_context(tc.tile_pool(name="oT", bufs=2))
        sp = ctx.enter_context(tc.tile_pool(name="small", bufs=1))
        m2p = ctx.enter_context(tc.tile_pool(name="m2", bufs=1))
        yp = ctx.enter_context(tc.tile_pool(name="yst", bufs=2))
        ps_g = ctx.enter_context(tc.tile_pool(name="psg", bufs=2,
                                              space="PSUM"))
        ps_a = ctx.enter_context(tc.tile_pool(name="psa", bufs=1,
                                              space="PSUM"))

        # ---- persistent weights (qAct DMA queue) ----
        qkvw_t = wp.tile([128, 4, 3 * C], BF16, tag="qkvw")
        nc.scalar.dma_start(qkvw_t[:],
                            qkvw[:].rearrange("(a p) k -> p a k", p=128))
        pw_t = wp.tile([128, 4, C], BF16, tag="pwt")
        rw_t = wp.tile([128, 4, NW], F32, tag="rw")
        nc.sync.dma_start(rw_t[:], rw[:].rearrange("(a p) k -> p a k", p=128))
        rb_t = wp.tile([1, NW], F32, tag="rb")
        nc.sync.dma_start(rb_t[:], rb[:])
        rb64 = wp.tile([1, NW], F32, tag="rb64")
        nc.vector.tensor_scalar_mul(rb64[:], rb_t[:], 64.0)
        ones1 = wp.tile([1, NW], F32, tag="ones1")
        nc.vector.memset(ones1[:], 1.0)
        pbp = wp.tile([128, 4], F32, tag="pbp")
        nc.sync.dma_start(pbp[:], pb[:].rearrange("o (a p) -> (o p) a", p=128))
        expb2 = wp.tile([128, HEADS * NW], BF16, tag="expb2")
        iden49 = wp.tile([NW, NW], BF16, tag="iden49")
        make_identity(nc, iden49[:])
        consts = wp.tile([128, 260], BF16, tag="consts")
        pick_g = [consts[:, 0:128], consts[:, 128:256]]
        bones2 = consts[:, 256:258]

        # persistent PSUM tiles; gap regions pre-set once (off-DVE queues)
        qk_ps = [ps_a.tile([128, 512], F32, tag=f"qkps{p}", name=f"qkps{p}")
                 for p in range(2)]
        for p in range(2):
            nc.scalar.memzero(qk_ps[p][:])
        o_ps = [ps_a.tile([128, 512], F32, tag=f"ops{p}", name=f"ops{p}")
                for p in range(3)]
        den_ps = ps_a.tile([128, 512], F32, tag="denps")
        nc.scalar.memzero(den_ps[:])
        nc.scalar.add(den_ps[:], den_ps[:], 1.0)
        # block-diag A tiles (AV moving operand), gaps zeroed once
        a_m3 = [ap_.tile([128, 784], BF16, tag=f"am{p}", name=f"am{p}")
                for p in range(3)]
        for p in range(3):
            nc.gpsimd.memset(a_m3[p][:], 0.0)
        # persistent V tiles (double-buffered over i); zeroed once
        vt_tiles = [[vtp.tile([128, C], BF16, tag=f"vt{p}_{j}",
                              name=f"vt{p}_{j}") for j in range(8)]
                    for p in range(2)]
        for p in range(2):
            for j in range(8):
                nc.gpsimd.memset(vt_tiles[p][j][:], 0.0)
        maskT2 = wp.tile([128, NW], BF16, tag="maskT2")
        nc.gpsimd.memset(maskT2[:], 0.0)
        m2t = [m2p.tile([128, NW], BF16, tag=f"m2t{h}", name=f"m2t{h}")
               for h in range(HEADS)]

        # ---------------- emission helpers ----------------
        def emit_x_loads(bb):
            xt = [xp.tile([128, HW], BF16, tag=f"xb{c}", name=f"xb{c}")
                  for c in range(4)]
            xr = [xrp.tile([128, HW], BF16, tag="xr", name="xr")
                  for c in range(4)]
            for c in range(4):
                nc.scalar.dma_start(xr[c][:],
                                    xbd[bb, c * 128:(c + 1) * 128, :])
                # window-ordered x3: col = (8*th+tw)*49 + 7a + b
                xsv = xr[c][:].rearrange("p (a t b u) -> p t u a b",
                                         a=7, t=8, b=7, u=8)
                x3v = xt[c][:].rearrange("p (t u a b) -> p t u a b",
                                         t=8, u=8, a=7, b=7)
                eng = nc.gpsimd if c % 2 == 0 else nc.vector
                eng.tensor_copy(x3v, xsv)
            return xt

        def emit_pooling(bb):
            xp_t = sp.tile([128, 4, NW], F32, tag="xpool", name="xpool")
            for c in range(4):
                xstage = xsp.tile([128, HW], F32, tag="xstage", name="xs")
                nc.sync.dma_start(xstage[:],
                                  xd[bb, c * 128:(c + 1) * 128, :])
                s1 = xsp.tile([128, 56, 7], F32, tag="pool1", name="s1")
                v = xstage[:].rearrange("p (h b u) -> p h b u",
                                        h=56, b=7, u=8)
                nc.vector.reduce_sum(s1[:], v, axis=AX.X)
                v2 = s1[:].rearrange("p (a t) b -> p a b t", a=7, t=8)
                nc.vector.reduce_sum(xp_t[:, c, :], v2, axis=AX.X)
            return xp_t

        def emit_router(xp_t):
            ps_s = ps_g.tile([128, 512], F32, tag="g", name="ps_s")
            for c in range(4):
                nc.tensor.matmul(ps_s[0:NW, 0:NW], xp_t[:, c, :],
                                 rw_t[:, c, :], start=(c == 0), stop=False)
            nc.tensor.matmul(ps_s[0:NW, 0:NW], ones1[:], rb64[:],
                             start=False, stop=True)
            s_sb = sp.tile([NW, NW], F32, tag="s_sb", name="ssb")
            nc.scalar.activation(s_sb[:], ps_s[0:NW, 0:NW], ACTF.Copy,
                                 scale=1.0 / 64.0)
            t8 = sp.tile([NW, 8], F32, tag="t8", name="t8")
            nc.vector.max(t8[:], s_sb[:])
            mask_bf = sp.tile([NW, NW], BF16, tag="mask", name="mask")
            with nc.allow_low_precision(reason="binary mask exact in bf16"):
                nc.vector.tensor_scalar(out=mask_bf[:], in0=s_sb[:],
                                        scalar1=t8[:, 3:4], scalar2=None,
                                        op0=AL.is_ge)
            mt_ps = ps_g.tile([128, 512], F32, tag="g", name="ps_mt")
            nc.tensor.matmul(mt_ps[0:NW, 0:NW], mask_bf[:], iden49[:],
                             start=True, stop=True)
            nc.scalar.copy(maskT2[0:NW, :], mt_ps[0:NW, 0:NW])
            nc.scalar.copy(maskT2[64:64 + NW, :], mt_ps[0:NW, 0:NW])
            for h in range(HEADS):
                hs = slice(h * NW, (h + 1) * NW)
                nc.vector.tensor_tensor(out=m2t[h][:], in0=expb2[:, hs],
                                        in1=maskT2[:], op=AL.mult)

        def new_qkt():
            return [qkp.tile([128, 2, 392], BF16, tag=f"qkt{m}",
                             name=f"qkt{m}") for m in range(8)]

        def emit_qk_gemm(i, xt, qkt, ms):
            for m in ms:
                for g in range(2):
                    th = 2 * i + g
                    ps = ps_g.tile([128, 512], F32, tag="g",
                                   name=f"psqk{m}_{g}")
                    for c in range(4):
                        nc.tensor.matmul(
                            ps[:, 0:392],
                            qkvw_t[:, c, m * 128:(m + 1) * 128],
                            xt[c][:, th * 392:(th + 1) * 392],
                            start=(c == 0), stop=(c == 3))
                    if m % 2 == 0:
                        nc.vector.tensor_copy(qkt[m][:, g, :], ps[:, 0:392])
                    else:
                        nc.scalar.copy(qkt[m][:, g, :], ps[:, 0:392])

        def emit_v_gemm(i, xt, jvs):
            vt = vt_tiles[i % 2]
            for jv in jvs:
                g, v = jv // 4, jv % 4
                ps = ps_g.tile([128, 512], F32, tag="g", name=f"psv{jv}")
                for c in range(4):
                    base = (2 * i + g) * 392 + 2 * v * NW
                    nc.tensor.matmul(ps[0:98, :],
                                     xt[c][:, base:base + 98],
                                     qkvw_t[:, c, 2 * C:3 * C],
                                     start=(c == 0), stop=(c == 3))
                vtmp = vsp.tile([98, C], BF16, tag="vtmp", name="vtmp")
                nc.scalar.copy(vtmp[:], ps[0:98, :])
                nc.sync.dma_start(vt[jv][0:49, :], vtmp[0:49, :])
                nc.sync.dma_start(vt[jv][64:113, :], vtmp[49:98, :])

        def emit_attention_ct(i, ct, qkt, of):
            vt = vt_tiles[i % 2]
            ob = [o_ps[(2 * ct + g) % 3] for g in range(2)]
            for hp in range(4):
                h = 4 * ct + hp
                qps = qk_ps[hp % 2]
                rs = slice(32 * hp, 32 * hp + 32)
                for g in range(2):
                    for tw in range(8):
                        cl = slice(tw * NW, (tw + 1) * NW)
                        band = 64 * (tw % 2)
                        dcol = 196 * g + NW * (tw // 2)
                        nc.tensor.matmul(
                            qps[band:band + NW, dcol:dcol + NW],
                            qkt[4 + ct][rs, g, cl], qkt[ct][rs, g, cl],
                            start=True, stop=True, skip_group_check=True,
                            tile_position=(32 * hp, band))
                a_sb = ap_.tile([128, 392], BF16, tag="a_sb", name="a_sb")
                nc.scalar.activation(a_sb[:], qps[:, 0:392], ACTF.Exp,
                                     scale=SCALE)
                # dense masked tile (one 4x TT over all 16 tiles)
                a_d = ap_.tile([128, 392], BF16, tag="a_d", name="a_d")
                nc.vector.tensor_tensor(
                    out=a_d[:].rearrange("p (r c) -> p r c", r=8),
                    in0=a_sb[:].rearrange("p (r c) -> p r c", r=8),
                    in1=m2t[h][:].unsqueeze(1).broadcast_to([128, 8, NW]),
                    op=AL.mult)
                # single den matmul: rows (32hp, 32hp+1) = (band0, band1)
                nc.tensor.matmul(
                    den_ps[32 * hp:32 * hp + 2, 0:392], bones2[:],
                    a_d[:], start=True, stop=True,
                    skip_group_check=True, tile_position=(0, 32 * hp))
                # scatter dense -> block-diag placement (4x copies)
                am = a_m3[(4 * ct + hp) % 3]
                vm = am[:].rearrange("p (g v c) -> p g v c", g=2, v=4, c=98)
                vd = a_d[:].rearrange("p (g w c) -> p g w c", g=2, w=4, c=49)
                nc.vector.tensor_copy(vm[0:NW, :, :, 0:NW], vd[0:NW])
                nc.vector.tensor_copy(vm[64:64 + NW, :, :, NW:2 * NW],
                                      vd[64:64 + NW])
                for g in range(2):
                    for v in range(4):
                        nc.tensor.matmul(
                            ob[g][rs, 98 * v:98 * v + 98],
                            vt[4 * g + v][:, h * 32:(h + 1) * 32],
                            am[:, g * 392 + 98 * v:g * 392 + 98 * v + 98],
                            start=True, stop=True, skip_group_check=True,
                            tile_position=(0, 32 * hp))
            # normalize for this ct
            r_den = rp.tile([128, 392], BF16, tag="r_den", name="r_den")
            with nc.allow_low_precision(reason="recip den bf16 ok"):
                nc.vector.reciprocal(r_den[:], den_ps[:, 0:392])
            for g in range(2):
                # rD cols (tw*49+q): parity par of tw picks den row
                # 32*(m//32)+par, value at dense col 196*g + 49*(tw//2) + q
                for par in range(2):
                    rv = r_den[:, 196 * g:196 * (g + 1)]
                    nc.tensor.matmul(
                        den_ps[:, 196 * par:196 * (par + 1)],
                        pick_g[par], rv,
                        start=True, stop=True, skip_group_check=True)
                rD = rp.tile([128, 392], BF16, tag="rD", name="rD")
                # interleave parity halves: rD col (2w+par)*49+c <-
                # ex col par*196 + 49w + c
                div = rD[:].rearrange("p (v d c) -> p d v c", v=4, d=2)
                src_v = den_ps[:, 0:392].rearrange("p (d w c) -> p d w c",
                                                   d=2, w=4)
                if g == 0:
                    nc.scalar.copy(div, src_v)
                else:
                    nc.vector.tensor_copy(div, src_v)
                nc.vector.tensor_tensor(out=of[ct][g][:],
                                        in0=ob[g][:, 0:392],
                                        in1=rD[:], op=AL.mult)

        def emit_proj_mo(bb, i, of, mo):
            for g in range(2):
                th = 2 * i + g
                ps = ps_g.tile([128, 512], F32, tag="g", name=f"psy{mo}_{g}")
                for ct in range(4):
                    nc.tensor.matmul(
                        ps[:, 0:392], pw_t[:, ct, mo * 128:(mo + 1) * 128],
                        of[ct][g][:], start=(ct == 0), stop=(ct == 3))
                yst = yp.tile([128, 392], F32, tag="yst", name="yst")
                yv = yst[:].rearrange("p (a b tw) -> p tw a b",
                                      a=7, b=7, tw=8)
                nc.scalar.activation(
                    yv, ps[:, 0:392].rearrange("p (tw a b) -> p tw a b",
                                               tw=8, a=7),
                    ACTF.Identity, bias=pbp[:, mo:mo + 1])
                ydv = yd[bb, mo * 128:(mo + 1) * 128, :].rearrange(
                    "p (a t r) -> p t a r", a=7, t=8, r=56)
                nc.scalar.dma_start(
                    ydv[:, th:th + 1],
                    yst[:].rearrange("p (a r) -> p a r", a=7))

        # ---------------- pipelined emission ----------------
        stages = [(bb, i) for bb in range(nb) for i in range(4)]
        xt_cur = emit_x_loads(0)
        xp_t = emit_pooling(0)
        # deferred weight loads (behind qkvw + x on the Act queue)
        nc.scalar.dma_start(consts[:], cbd[:])
        nc.scalar.dma_start(expb2[:], ebd[:])
        nc.scalar.dma_start(pw_t[:],
                            pw[:].rearrange("(a p) k -> p a k", p=128))
        qkt_cur = new_qkt()
        emit_v_gemm(0, xt_cur, range(8))
        emit_qk_gemm(0, xt_cur, qkt_cur, [0, 4, 1, 5, 2, 6, 3, 7])
        emit_router(xp_t)

        prev = None          # (bb, i, of)
        xt_next = None
        for s, (bb, i) in enumerate(stages):
            nxt = stages[s + 1] if s + 1 < len(stages) else None
            of = [[op_.tile([128, 392], BF16, tag=f"of{ct}_{g}",
                            name=f"of{ct}_{g}") for g in range(2)]
                  for ct in range(4)]
            qkt_next = new_qkt() if nxt else None
            for ct in range(4):
                emit_attention_ct(i, ct, qkt_cur, of)
                if prev is not None:
                    emit_proj_mo(prev[0], prev[1], prev[2], ct)
                if i == 2 and bb + 1 < nb and ct == 1:
                    # prefetch next image early
                    xt_next = emit_x_loads(bb + 1)
                if i == 2 and bb + 1 < nb and ct == 3:
                    xp_t = emit_pooling(bb + 1)
                if nxt is not None:
                    nb_, ni = nxt
                    nxt_xt = xt_cur if nb_ == bb else xt_next
                    if ct == 0:
                        emit_v_gemm(ni, nxt_xt, range(0, 4))
                    elif ct == 1:
                        emit_v_gemm(ni, nxt_xt, range(4, 8))
                    elif ct == 2:
                        emit_qk_gemm(ni, nxt_xt, qkt_next, [0, 4, 1, 5])
                    else:
                        emit_qk_gemm(ni, nxt_xt, qkt_next, [2, 6, 3, 7])
                        if nb_ != bb:
                            emit_router(xp_t)
            prev = (bb, i, of)
            qkt_cur = qkt_next
            if nxt is not None and nxt[0] != bb:
                xt_cur = xt_next
        for mo in range(4):
            emit_proj_mo(prev[0], prev[1], prev[2], mo)

    nc.compile()
    return nc


def _rel_index(n):
    coords = np.stack(np.meshgrid(np.arange(n), np.arange(n), indexing="ij"),
                      0).reshape(2, -1)
    rel = (coords[:, :, None] - coords[:, None, :]).transpose(1, 2, 0)
    rel[..., 0] += n - 1
    rel[..., 1] += n - 1
    rel[..., 0] *= 2 * n - 1
    return rel.sum(-1)


def host_prep(x, router_w, router_b, qkv_w, proj_w, proj_b, rpb_table):
    import ml_dtypes
    BF = ml_dtypes.bfloat16
    x = np.ascontiguousarray(np.asarray(x, np.float32).reshape(B_FULL, C, HW))
    x_bf = x.astype(BF)
    rel = _rel_index(NWIN_SIDE)
    bias = np.asarray(rpb_table, np.float32)[rel]        # (49 q, 49 k, 16 h)
    eb = np.exp(bias).transpose(1, 2, 0).reshape(NW, HEADS * NW)  # k,(h,q)
    expbT2 = np.zeros((128, HEADS * NW), np.float32)
    expbT2[0:NW] = eb
    expbT2[64:64 + NW] = eb
    # consts: pick_g0 [128,128], pick_g1 [128,128], bones_g0/g1 [128,2]
    consts = np.zeros((128, 260), np.float32)
    for g in range(2):
        for q in range(4):
            consts[32 * q + g, g * 128 + 32 * q:g * 128 + 32 * q + 32] = 1.0
    consts[0:NW, 256] = 1.0
    consts[64:64 + NW, 257] = 1.0
    shared = {
        "qkv_wTb": np.ascontiguousarray(
            np.asarray(qkv_w, np.float32).T).astype(BF),
        "r_wT": np.ascontiguousarray(np.asarray(router_w, np.float32).T),
        "router_b": np.ascontiguousarray(
            np.asarray(router_b, np.float32).reshape(1, NW)),
        "proj_wTb": np.ascontiguousarray(
            np.asarray(proj_w, np.float32).T).astype(BF),
        "proj_b": np.ascontiguousarray(
            np.asarray(proj_b, np.float32).reshape(1, C)),
        "expbT2": expbT2.astype(BF),
        "constsb": consts.astype(BF),
    }
    in_maps = []
    for core in range(N_CORES):
        m = dict(shared)
        m["x"] = np.ascontiguousarray(x[core * NB:(core + 1) * NB])
        m["x_bf"] = np.ascontiguousarray(x_bf[core * NB:(core + 1) * NB])
        in_maps.append(m)
    return in_maps


_NC_CACHE = {}


def _get_nc():
    if "nc" not in _NC_CACHE:
        _NC_CACHE["nc"] = build_nc2(NB)
    return _NC_CACHE["nc"]


def kernel(x, router_w, router_b, qkv_w, proj_w, proj_b, rpb_table):
    in_maps = host_prep(x, router_w, router_b, qkv_w, proj_w, proj_b,
                        rpb_table)
    nc = _get_nc()
    res = run_bass_kernel_spmd(nc, in_maps, core_ids=list(range(N_CORES)))
    ys = [res.results[i]["y"] for i in range(N_CORES)]
    y = np.concatenate(ys, axis=0).reshape(B_FULL, C, H, W)
    return y.astype(np.float32)

